# revision 1
# baseline (speedup 1.0000x reference)
"""Trainium2 Bass kernel for nn_DLKAConvBlock (B=4, C=64, H=W=256) on 8 NeuronCores.

Sharding: data-parallel over (batch, H-half): core = 2*b + half, each core
computes output rows [r0, r0+128) of image b (r0 = 128*half), working in a
local row coordinate frame l (img row = r0 + l) so the SPMD program is
identical across cores; all per-core differences are carried by input data
(host-shifted, zero-padded slices + row-validity masks).

Pipeline per core (all on-device):
  conv3x3 full image (stats only) -> instance-norm scale/bias
  conv3x3 on local rows -> h_local
  t = gelu(p1 @ norm(h)) (masked to valid rows) -> t_nchw + t_pad (NHWC, padded)
  off0 = conv5x5(t); deformable-depthwise-5x5 via SWDGE dma_gather of
  2x1-pixel pairs (2 rows per tap) + DVE bilinear combine + PE
  transpose-accumulate over taps -> a1 (rows [-16,144))
  offs = conv7x7-dil3(a1); deformable-depthwise-7x7 similarly -> a2
  tail: g1, u*a, p2, +shortcut, leaky-relu -> out rows [0,128)
"""
import os
import sys
from contextlib import ExitStack

import numpy as np

for _p in ("/opt/trn_rl_repo", "/root/.axon_site/_ro/trn_rl_repo"):
    if os.path.isdir(_p) and _p not in sys.path:
        sys.path.insert(0, _p)

import concourse.bass as bass
import concourse.bacc as bacc
import concourse.mybir as mybir
from concourse import tile
from concourse.bass_utils import run_bass_kernel_spmd

F32 = mybir.dt.float32
I16 = mybir.dt.int16
ALU = mybir.AluOpType
ACTF = mybir.ActivationFunctionType
F32R = mybir.dt.float32r

B, C, H, W = 4, 64, 256, 256
EPS = 1e-5
N_CORES = 8

# local-frame regions
HL0, HL1 = -24, 152          # h_local / t rows
NHROWS = HL1 - HL0           # 176
A1L0, A1L1 = -16, 144        # a1 rows
NA1ROWS = A1L1 - A1L0        # 160
PADR = 32                    # nhwc row pad (array row = l + 32)
PADC = 16                    # nhwc col pad
NPR = 192                    # nhwc rows: l in [-32, 160)
NPC = 288                    # nhwc cols: x in [-16, 272)
K1, K2 = 25, 49

D1_CHUNKS = [(-16, 48), (48, 112), (112, 144)]
D2_CHUNKS = [(0, 64), (64, 128)]
WIN1 = 8    # deform1 window margin rows (reach = 2 + |off|<=4 + 1)
WIN2 = 16   # deform2 window margin rows (reach = 9 + |off|<=4 + 1)


def _ap_raw(t_handle, offset, pattern):
    """Build an AP with an explicit [step, count] pattern on a tensor handle."""
    return bass.AP(t_handle, offset, [list(p) for p in pattern])


def build_program():
    nc = bacc.Bacc("TRN2", target_bir_lowering=False, debug=False, enable_asserts=False)

    # ---------------- external inputs ----------------
    x_full = nc.declare_dram_parameter("x_full", [C, H + 2, W + 2], F32R, isOutput=False)
    x_local = nc.declare_dram_parameter("x_local", [C, NHROWS + 2, W + 2], F32R, isOutput=False)
    hmask = nc.declare_dram_parameter("hmask", [NHROWS // 2, C, 2 * W], F32, isOutput=False)
    cw3 = nc.declare_dram_parameter("cw3", [C, 9, C], F32R, isOutput=False)
    p1w = nc.declare_dram_parameter("p1w", [C, C], F32, isOutput=False)
    p1b = nc.declare_dram_parameter("p1b", [C, 1], F32, isOutput=False)
    off0w = nc.declare_dram_parameter("off0w", [C, K1, 2 * K1], F32R, isOutput=False)
    off0b = nc.declare_dram_parameter("off0b", [2 * K1, 1], F32, isOutput=False)
    offsw = nc.declare_dram_parameter("offsw", [C, K2, 2 * K2], F32R, isOutput=False)
    offsb = nc.declare_dram_parameter("offsb", [2 * K2, 1], F32, isOutput=False)
    dwk1 = nc.declare_dram_parameter("dwk1", [128, K1, C], F32, isOutput=False)
    dwk2 = nc.declare_dram_parameter("dwk2", [128, K2, C], F32, isOutput=False)
    g1w = nc.declare_dram_parameter("g1w", [C, C], F32, isOutput=False)
    g1b = nc.declare_dram_parameter("g1b", [C, 1], F32, isOutput=False)
    p2w = nc.declare_dram_parameter("p2w", [C, C], F32, isOutput=False)
    p2b = nc.declare_dram_parameter("p2b", [C, 1], F32, isOutput=False)
    identw = nc.declare_dram_parameter("identw", [128, 128], F32, isOutput=False)
    by1 = nc.declare_dram_parameter("by1", [128, 4 * K1], F32, isOutput=False)
    bx1 = nc.declare_dram_parameter("bx1", [128, 4 * K1], F32, isOutput=False)
    by2 = nc.declare_dram_parameter("by2", [128, 4 * K2], F32, isOutput=False)
    bx2 = nc.declare_dram_parameter("bx2", [128, 4 * K2], F32, isOutput=False)

    out_t = nc.declare_dram_parameter("out", [C, 128, W], F32, isOutput=True)

    # ---------------- internal DRAM ----------------
    h_local = nc.dram_tensor("h_local", [C, NHROWS, W], F32)
    t_nchw = nc.dram_tensor("t_nchw", [C, NHROWS, W + 4], F32R)
    t_pad = nc.dram_tensor("t_pad", [NPR, NPC, C], F32)
    a1_nchw = nc.dram_tensor("a1_nchw", [C, NA1ROWS, W + 18], F32R)
    a1_pad = nc.dram_tensor("a1_pad", [NPR, NPC, C], F32)
    NSCR = 4
    scr1 = nc.dram_tensor("scr1", [NSCR, 128, 8 * K1], I16)
    scr2 = nc.dram_tensor("scr2", [NSCR, 128, 8 * K2], I16)

    with tile.TileContext(nc) as tc, ExitStack() as ctx:
        PHASES = int(os.environ.get("KERNEL_PHASES", "5"))
        gather_regs = {n: nc.gpsimd.to_reg(n)
                       for n in (2 * K1 * 128, 25 * 2 * 128, 24 * 2 * 128)}
        statics = ctx.enter_context(tc.tile_pool(name="statics", bufs=1))
        # resident static tiles
        s_cw3 = statics.tile([C, 9, C], F32R)
        s_p1w = statics.tile([C, C], F32)
        s_p1b = statics.tile([C, 1], F32)
        s_g1w = statics.tile([C, C], F32)
        s_g1b = statics.tile([C, 1], F32)
        s_p2w = statics.tile([C, C], F32)
        s_p2b = statics.tile([C, 1], F32)
        s_id = statics.tile([128, 128], F32)
        s_zero = statics.tile([128, 1024], F32)
        for dst, src in [(s_cw3, cw3), (s_p1w, p1w), (s_p1b, p1b),
                         (s_g1w, g1w), (s_g1b, g1b), (s_p2w, p2w), (s_p2b, p2b),
                         (s_id, identw)]:
            nc.sync.dma_start(dst[:], src[:])
        nc.vector.memset(s_zero[:], 0.0)

        # stats accumulators
        s_sum = statics.tile([C, 128], F32)
        s_sq = statics.tile([C, 128], F32)
        s_rstd = statics.tile([C, 1], F32)
        s_nbias = statics.tile([C, 1], F32)   # -mean*rstd
        s_cb = statics.tile([C, 1], F32)      # p2b + nbias
        s_tmp1 = statics.tile([C, 1], F32)
        s_tmp2 = statics.tile([C, 1], F32)

        # ---------------- memset DRAM pads ----------------
        for dram in (t_nchw, t_pad, a1_nchw, a1_pad):
            flat = dram[:].bitcast(F32).rearrange("a b c -> (a b c)")
            total = int(np.prod(dram.shape))
            CH = 128 * 1024
            pos = 0
            while pos < total:
                n = min(CH, total - pos)
                rows = n // 1024
                if rows >= 1 and rows * 1024 == n:
                    nc.scalar.dma_start(
                        flat[pos:pos + n].rearrange("(p f) -> p f", p=rows),
                        s_zero[:rows, :])
                else:
                    nc.scalar.dma_start(flat[pos:pos + n], s_zero[0:1, :n])
                pos += n

        psum_conv = ctx.enter_context(
            tc.tile_pool(name="psum_conv", bufs=2, space="PSUM"))
        psum_tr = ctx.enter_context(
            tc.tile_pool(name="psum_tr", bufs=2, space="PSUM"))
        psum_acc = ctx.enter_context(
            tc.tile_pool(name="psum_acc", bufs=2, space="PSUM"))
        psum_tail = ctx.enter_context(
            tc.tile_pool(name="psum_tail", bufs=2, space="PSUM"))

        # ---------------- phase 1: full-image conv3x3 stats ----------------
        with tc.tile_pool(name="ph1", bufs=3) as ph1:
          if PHASES >= 1:
            for g in range(H // 2):
                xt = ph1.tile([C, 4, W + 2], F32R, tag="xt")
                nc.sync.dma_start(xt[:], x_full[:, 2 * g:2 * g + 4, :])
                ps = psum_conv.tile([C, 2 * W], F32, tag="conv")
                for k in range(9):
                    ky, kx = k // 3, k % 3
                    rhs = xt[:, ky:ky + 2, kx:kx + W]
                    nc.tensor.matmul(ps[:].rearrange("c (r w) -> c r w", r=2),
                                     s_cw3[:, k, :], rhs,
                                     start=(k == 0), stop=(k == 8))
                hd = ph1.tile([C, 2 * W], F32, tag="hd")
                nc.scalar.activation(hd[:], ps[:], ACTF.Copy,
                                     accum_out=s_sum[:, g:g + 1])
                sqd = ph1.tile([C, 2 * W], F32, tag="sqd")
                nc.scalar.activation(sqd[:], hd[:], ACTF.Square,
                                     accum_out=s_sq[:, g:g + 1])

        # finalize stats
        nc.vector.tensor_reduce(s_tmp1[:], s_sum[:], mybir.AxisListType.X, ALU.add)
        nc.vector.tensor_reduce(s_tmp2[:], s_sq[:], mybir.AxisListType.X, ALU.add)
        inv_n = 1.0 / (H * W)
        # mean -> s_tmp1, E[x^2] -> s_tmp2
        nc.vector.tensor_scalar(s_tmp1[:], s_tmp1[:], inv_n, None, ALU.mult)
        nc.vector.tensor_scalar(s_tmp2[:], s_tmp2[:], inv_n, None, ALU.mult)
        # var = E[x^2] - mean^2 ; rstd = 1/sqrt(var+eps)
        var = statics.tile([C, 1], F32)
        # (mean*mean) - E[x^2] = -var ; then negate and add eps
        nc.vector.scalar_tensor_tensor(var[:], s_tmp1[:], s_tmp1[:], s_tmp2[:],
                                       ALU.mult, ALU.subtract)
        nc.vector.tensor_scalar(var[:], var[:], -1.0, EPS, ALU.mult, ALU.add)
        nc.scalar.sqrt(var[:], var[:])
        nc.vector.reciprocal(s_rstd[:], var[:])
        nc.vector.scalar_tensor_tensor(s_nbias[:], s_tmp1[:], -1.0, s_rstd[:],
                                       ALU.mult, ALU.mult)
        nc.vector.tensor_tensor(s_cb[:], s_p2b[:], s_nbias[:], ALU.add)

        # ---------------- phase 2: h_local conv3x3 ----------------
        with tc.tile_pool(name="ph2", bufs=3) as ph2:
          if PHASES >= 2:
            for g in range(NHROWS // 2):
                xt = ph2.tile([C, 4, W + 2], F32R, tag="xt")
                nc.sync.dma_start(xt[:], x_local[:, 2 * g:2 * g + 4, :])
                ps = psum_conv.tile([C, 2 * W], F32, tag="conv")
                for k in range(9):
                    ky, kx = k // 3, k % 3
                    nc.tensor.matmul(ps[:].rearrange("c (r w) -> c r w", r=2),
                                     s_cw3[:, k, :], xt[:, ky:ky + 2, kx:kx + W],
                                     start=(k == 0), stop=(k == 8))
                hsb = ph2.tile([C, 2 * W], F32, tag="hsb")
                nc.scalar.activation(hsb[:], ps[:], ACTF.Copy)
                nc.scalar.dma_start(h_local[:, 2 * g:2 * g + 2, :],
                                    hsb[:].rearrange("c (r w) -> c r w", r=2))

        # ---------------- phase 3: t = mask*gelu(p1 @ norm(h)) ----------------
        with tc.tile_pool(name="ph3", bufs=3) as ph3:
          if PHASES >= 3:
            for g in range(NHROWS // 2):
                hsb = ph3.tile([C, 2 * W], F32, tag="hld")
                nc.sync.dma_start(
                    hsb[:], h_local[:, 2 * g:2 * g + 2, :].rearrange("c r w -> c (r w)"))
                hn = ph3.tile([C, 2 * W], F32, tag="hn")
                nc.vector.tensor_scalar(hn[:], hsb[:], s_rstd[:], s_nbias[:],
                                        ALU.mult, ALU.add)
                ps = psum_conv.tile([C, 2 * W], F32, tag="conv")
                nc.tensor.matmul(ps[:], s_p1w[:], hn[:], start=True, stop=True)
                tt_ = ph3.tile([C, 2 * W], F32, tag="tt")
                nc.scalar.activation(tt_[:], ps[:], ACTF.Gelu, bias=s_p1b[:])
                mk = ph3.tile([C, 2 * W], F32, tag="mk")
                nc.sync.dma_start(mk[:], hmask[g, :, :])
                tm = ph3.tile([C, 2 * W], F32, tag="tm")
                nc.vector.tensor_tensor(tm[:], tt_[:], mk[:], ALU.mult)
                nc.scalar.dma_start(t_nchw[:, 2 * g:2 * g + 2, 2:2 + W].bitcast(F32),
                                    tm[:].rearrange("c (r w) -> c r w", r=2))
                # NHWC transposed copies
                for bb in range(4):
                    pst_full = psum_tr.tile([128, 128], F32, tag="tr")
                    pst = pst_full[:, :C]
                    nc.tensor.matmul(pst[:], tm[:, 128 * bb:128 * (bb + 1)],
                                     s_id[:C, :C], start=True, stop=True,
                                     is_transpose=True)
                    tT = ph3.tile([128, C], F32, tag="tT")
                    nc.vector.tensor_copy(tT[:], pst[:])
                    l = 2 * g + HL0 + bb // 2
                    xh = bb % 2
                    nc.scalar.dma_start(
                        t_pad[l + PADR, PADC + 128 * xh: PADC + 128 * (xh + 1), :],
                        tT[:])

        # ---------------- deform stages ----------------
        def deform_stage(name, chunks, KK, d_by, d_bx, d_offw, d_offb, d_dwk,
                         src_nchw, src_pad, scr, win_margin, kspan, out_stage):
            """Emit one deformable depthwise conv stage.

            out_stage(l0, rp, xh, acc_psum, dpool) consumes the [C,128] tap-sum."""
            Kg2 = 2 * KK
            with tc.tile_pool(name=name + "s", bufs=1) as st, \
                 tc.tile_pool(name=name, bufs=2) as dp, \
                 tc.tile_pool(name=name + "r", bufs=2) as rp_pool, \
                 tc.tile_pool(name=name + "g", bufs=2) as gp:
                s_offw = st.tile([C, KK, Kg2], F32R)
                s_offb = st.tile([Kg2, 1], F32)
                s_dwk = st.tile([128, KK, C], F32)
                s_bw = st.tile([128, 4 * KK], F32)
                s_bxw = st.tile([128, 4 * KK], F32)
                for dst, src in [(s_offw, d_offw), (s_offb, d_offb),
                                 (s_dwk, d_dwk), (s_bw, d_by), (s_bxw, d_bx)]:
                    nc.sync.dma_start(dst[:], src[:])
                gidx = 0
                for (c0, c1) in chunks:
                    win_l0 = c0 - win_margin
                    win_rows = (c1 - c0) + 2 * win_margin
                    n_elems = win_rows * NPC
                    for l0 in range(c0, c1, 2):
                        # ---- offset conv on rows (l0, l0+1) ----
                        if name == "d1":
                            rt = rp_pool.tile([C, 6, W + 4], F32R, tag="rt")
                            nc.sync.dma_start(
                                rt[:], src_nchw[:, (l0 - 2) - HL0:(l0 + 4) - HL0, :])
                            ps = psum_conv.tile([Kg2, 2 * W], F32, tag="conv")
                            for k in range(KK):
                                ky, kx = k // 5 - 2, k % 5 - 2
                                nc.tensor.matmul(
                                    ps[:].rearrange("c (r w) -> c r w", r=2),
                                    s_offw[:, k, :],
                                    rt[:, (ky + 2):(ky + 4), (kx + 2):(kx + 2) + W],
                                    start=(k == 0), stop=(k == KK - 1))
                        else:
                            rt = rp_pool.tile([C, 20, W + 18], F32R, tag="rt")
                            nc.sync.dma_start(
                                rt[:], src_nchw[:, (l0 - 9) - A1L0:(l0 + 11) - A1L0, :])
                            ps = psum_conv.tile([Kg2, 2 * W], F32, tag="conv")
                            for k in range(KK):
                                ky, kx = 3 * (k // 7 - 3), 3 * (k % 7 - 3)
                                nc.tensor.matmul(
                                    ps[:].rearrange("c (r w) -> c r w", r=2),
                                    s_offw[:, k, :],
                                    rt[:, (ky + 9):(ky + 11), (kx + 9):(kx + 9) + W],
                                    start=(k == 0), stop=(k == KK - 1))
                        osb = dp.tile([Kg2, 2 * W], F32, tag="osb")
                        nc.scalar.activation(osb[:], ps[:], ACTF.Identity, bias=s_offb[:])
                        # ---- transpose offsets to [px, ch] for 4 batches ----
                        offsT = dp.tile([128, 4 * Kg2], F32, tag="offsT")
                        for bb in range(4):
                            pst_full = psum_tr.tile([128, 128], F32, tag="tr")
                            pst = pst_full[:, :Kg2]
                            nc.tensor.matmul(pst[:], osb[:, 128 * bb:128 * (bb + 1)],
                                             s_id[:Kg2, :Kg2], start=True, stop=True,
                                             is_transpose=True)
                            nc.vector.tensor_copy(
                                offsT[:, Kg2 * bb:Kg2 * (bb + 1)], pst[:])
                        # ---- index & weight prep on [128, 4*KK] views ----
                        yv = offsT[:].rearrange("p (b c) -> p b c", b=4)[:, :, 0:KK]
                        xv = offsT[:].rearrange("p (b c) -> p b c", b=4)[:, :, KK:Kg2]
                        py = dp.tile([128, 4 * KK], F32, tag="py")
                        px = dp.tile([128, 4 * KK], F32, tag="px")
                        pyv = py[:].rearrange("p (b k) -> p b k", b=4)
                        pxv = px[:].rearrange("p (b k) -> p b k", b=4)
                        nc.vector.tensor_tensor(pyv, yv, s_bw[:].rearrange(
                            "p (b k) -> p b k", b=4), ALU.add)
                        nc.vector.tensor_scalar(py[:], py[:], float(l0 - win_l0),
                                                None, ALU.add)
                        nc.vector.tensor_tensor(pxv, xv, s_bxw[:].rearrange(
                            "p (b k) -> p b k", b=4), ALU.add)
                        # y0 = round(py - 0.5) via the fp32 magic-number trick;
                        # equals floor(py) except exact-integer ties, which
                        # still yield a valid (y0, fy=py-y0) bilinear pair.
                        MAGIC = 8388608.0
                        y0 = dp.tile([128, 4 * KK], F32, tag="y0")
                        x0 = dp.tile([128, 4 * KK], F32, tag="x0")
                        nc.vector.tensor_scalar(y0[:], py[:], MAGIC - 0.5,
                                                -MAGIC, ALU.add, ALU.add)
                        nc.vector.tensor_scalar(x0[:], px[:], MAGIC - 0.5,
                                                -MAGIC, ALU.add, ALU.add)
                        fy = dp.tile([128, 4 * KK], F32, tag="fy")
                        fx = dp.tile([128, 4 * KK], F32, tag="fx")
                        nc.vector.tensor_tensor(fy[:], py[:], y0[:], ALU.subtract)
                        nc.vector.tensor_tensor(fx[:], px[:], x0[:], ALU.subtract)
                        nc.vector.tensor_scalar(y0[:], y0[:], float(win_rows - 2),
                                                0.0, ALU.min, ALU.max)
                        nc.vector.tensor_scalar(x0[:], x0[:], float(NPC - 2),
                                                0.0, ALU.min, ALU.max)
                        idxf = dp.tile([128, 4 * KK], F32, tag="idxf")
                        nc.vector.scalar_tensor_tensor(idxf[:], y0[:], float(NPC),
                                                       x0[:], ALU.mult, ALU.add)
                        idxa = dp.tile([128, 4 * Kg2], I16, tag="idxa")
                        iv = idxa[:].rearrange("p (b k d) -> p (b k) d", b=4, d=2)
                        nc.vector.tensor_copy(iv[:, :, 0:1].rearrange("p k d -> p (k d)"),
                                              idxf[:])
                        nc.vector.tensor_scalar(
                            iv[:, :, 1:2].rearrange("p k d -> p (k d)"),
                            iv[:, :, 0:1].rearrange("p k d -> p (k d)"),
                            NPC, None, ALU.add)
                        fyb = dp.tile([128, 4 * KK], F32, tag="fyb")
                        fxb = dp.tile([128, 4 * KK], F32, tag="fxb")
                        nc.vector.tensor_scalar(fyb[:], fy[:], -1.0, 1.0,
                                                ALU.mult, ALU.add)
                        nc.vector.tensor_scalar(fxb[:], fx[:], -1.0, 1.0,
                                                ALU.mult, ALU.add)
                        w00 = dp.tile([128, 4 * KK], F32, tag="w00")
                        w01 = dp.tile([128, 4 * KK], F32, tag="w01")
                        w10 = dp.tile([128, 4 * KK], F32, tag="w10")
                        w11 = dp.tile([128, 4 * KK], F32, tag="w11")
                        nc.vector.tensor_tensor(w00[:], fyb[:], fxb[:], ALU.mult)
                        nc.vector.tensor_tensor(w01[:], fyb[:], fx[:], ALU.mult)
                        nc.vector.tensor_tensor(w10[:], fy[:], fxb[:], ALU.mult)
                        nc.vector.tensor_tensor(w11[:], fy[:], fx[:], ALU.mult)
                        # ---- rewrap indices via DRAM bounce ----
                        sb = scr[gidx % NSCR]
                        nc.sync.dma_start(sb[:, :], idxa[:])
                        wrapped = gp.tile([128, 4 * Kg2 * 8], I16, tag="wrp")
                        for rep in range(8):
                            nc.sync.dma_start(
                                wrapped[16 * rep:16 * (rep + 1), :].rearrange(
                                    "r (b g q) -> r b g q", b=4, g=Kg2),
                                sb[:].rearrange("(q r) (b g) -> r b g q",
                                                q=8, b=4))
                        gidx += 1
                        # ---- per batch: gather + combine ----
                        for bb in range(4):
                            rp, xh = bb // 2, bb % 2
                            win_off = (win_l0 + PADR) * NPC * C
                            gsets = ([(0, KK)] if KK == K1
                                     else [(0, 25), (25, KK)])
                            gtiles = []
                            for (ka, kb) in gsets:
                                ng = 2 * (kb - ka)
                                gt = gp.tile([128, 2 * 25, 128], F32, tag="gth")
                                inap = _ap_raw(
                                    src_pad, win_off, [[C, n_elems], [1, 2 * C]])
                                nc.gpsimd.dma_gather(
                                    gt[:, :ng, :],
                                    inap,
                                    wrapped[:, bb * Kg2 * 8 + ka * 16:
                                            bb * Kg2 * 8 + kb * 16],
                                    ng * 128, gather_regs[ng * 128], 2 * C, C,
                                    single_packet=False)
                                gtiles.append((ka, kb, gt))
                            acc = psum_acc.tile([C, 128], F32, tag="acc")
                            for (ka, kb, gt) in gtiles:
                                for k in range(ka, kb):
                                    gl = k - ka
                                    samp = dp.tile([128, C], F32, tag="samp")
                                    sw = dp.tile([128, C], F32, tag="sw")
                                    V00 = gt[:, 2 * gl, 0:C]
                                    V01 = gt[:, 2 * gl, C:2 * C]
                                    V10 = gt[:, 2 * gl + 1, 0:C]
                                    V11 = gt[:, 2 * gl + 1, C:2 * C]
                                    col = bb * KK + k
                                    nc.vector.tensor_scalar(
                                        samp[:], V00, w00[:, col:col + 1], None,
                                        ALU.mult)
                                    nc.vector.scalar_tensor_tensor(
                                        samp[:], V01, w01[:, col:col + 1], samp[:],
                                        ALU.mult, ALU.add)
                                    nc.vector.scalar_tensor_tensor(
                                        samp[:], V10, w10[:, col:col + 1], samp[:],
                                        ALU.mult, ALU.add)
                                    nc.vector.scalar_tensor_tensor(
                                        samp[:], V11, w11[:, col:col + 1], samp[:],
                                        ALU.mult, ALU.add)
                                    nc.vector.tensor_tensor(
                                        sw[:], samp[:], s_dwk[:, k, :], ALU.mult)
                                    nc.tensor.matmul(
                                        acc[:], sw[:], s_id[:],
                                        start=(k == 0), stop=(k == KK - 1),
                                        is_transpose=True)
                            out_stage(l0, rp, xh, acc, dp)

        # ---------------- deform1 consumer: write a1 ----------------
        def a1_out(l0, rp, xh, acc, dpool):
            l = l0 + rp
            a1sb = dpool.tile([C, 128], F32, tag="a1sb")
            nc.scalar.activation(a1sb[:], acc[:], ACTF.Copy)
            nc.scalar.dma_start(
                a1_nchw[:, l - A1L0,
                        9 + 128 * xh: 9 + 128 * (xh + 1)].bitcast(F32), a1sb[:])
            pst_full = psum_tr.tile([128, 128], F32, tag="tr")
            pst = pst_full[:, :C]
            nc.tensor.matmul(pst[:], a1sb[:], s_id[:C, :C], start=True, stop=True,
                             is_transpose=True)
            a1T = dpool.tile([128, C], F32, tag="a1T")
            nc.vector.tensor_copy(a1T[:], pst[:])
            nc.scalar.dma_start(
                a1_pad[l + PADR, PADC + 128 * xh: PADC + 128 * (xh + 1), :], a1T[:])

        def dump_to_out(src_dram, row_off, col_off, row_len):
            with tc.tile_pool(name="dump", bufs=2) as dmp:
                for g in range(64):
                    tl = dmp.tile([C, 2, W], F32, tag="dt")
                    nc.sync.dma_start(
                        tl[:], src_dram[:, row_off + 2 * g:row_off + 2 * g + 2,
                                        col_off:col_off + W].bitcast(F32))
                    nc.scalar.dma_start(out_t[:, 2 * g:2 * g + 2, :], tl[:])

        if PHASES >= 4:
            deform_stage("d1", D1_CHUNKS, K1, by1, bx1, off0w, off0b,
                         dwk1, t_nchw, t_pad, scr1, WIN1, 2, a1_out)

        # ---------------- deform2 consumer: tail fusion ----------------
        def tail_out(l0, rp, xh, acc, dpool):
            l = l0 + rp
            a2sb = dpool.tile([C, 128], F32, tag="a2sb")
            nc.scalar.activation(a2sb[:], acc[:], ACTF.Copy)
            psg = psum_tail.tile([C, 128], F32, tag="tail")
            nc.tensor.matmul(psg[:], s_g1w[:], a2sb[:], start=True, stop=True)
            ut = dpool.tile([C, 128], F32, tag="ut")
            nc.sync.dma_start(
                ut[:], t_nchw[:, l - HL0,
                              2 + 128 * xh: 2 + 128 * (xh + 1)].bitcast(F32))
            t2 = dpool.tile([C, 128], F32, tag="t2")
            nc.vector.scalar_tensor_tensor(t2[:], psg[:], s_g1b[:], ut[:],
                                           ALU.add, ALU.mult)
            psp = psum_tail.tile([C, 128], F32, tag="tail")
            nc.tensor.matmul(psp[:], s_p2w[:], t2[:], start=True, stop=True)
            ht = dpool.tile([C, 128], F32, tag="ht")
            nc.sync.dma_start(
                ht[:], h_local[:, l - HL0, 128 * xh: 128 * (xh + 1)])
            v1 = dpool.tile([C, 128], F32, tag="v1")
            nc.scalar.activation(v1[:], psp[:], ACTF.Identity, bias=s_cb[:])
            v2 = dpool.tile([C, 128], F32, tag="v2")
            nc.vector.scalar_tensor_tensor(v2[:], ht[:], s_rstd[:], v1[:],
                                           ALU.mult, ALU.add)
            v3 = dpool.tile([C, 128], F32, tag="v3")
            nc.vector.scalar_tensor_tensor(v3[:], v2[:], 0.2, v2[:],
                                           ALU.mult, ALU.max)
            nc.scalar.dma_start(out_t[:, l, 128 * xh: 128 * (xh + 1)], v3[:])

        if PHASES >= 5:
            deform_stage("d2", D2_CHUNKS, K2, by2, bx2, offsw, offsb,
                         dwk2, a1_nchw, a1_pad, scr2, WIN2, 6, tail_out)
        elif PHASES == 2:
            dump_to_out(h_local, -HL0, 0, 128)
        elif PHASES == 3:
            dump_to_out(t_nchw, -HL0, 2, 128)
        elif PHASES == 4:
            dump_to_out(a1_nchw, -A1L0, 9, 128)
        elif PHASES <= 1:
            dump_to_out(h_local, -HL0, 0, 128)

    nc.compile()
    return nc


def prepare_inputs(inputs):
    """Host-side marshaling: returns in_maps (list of 8 dicts)."""
    x = inputs["x"].astype(np.float32)
    conv_w = inputs["conv_w"].astype(np.float32)

    def reorder(idx_list):
        return np.array(idx_list, dtype=np.int64)

    # conv3x3 lhsT per shift: [cin, 9, cout]
    cw3 = np.ascontiguousarray(conv_w.transpose(1, 2, 3, 0).reshape(C, 9, C))
    p1w = np.ascontiguousarray(inputs["p1_w"].T)
    p1b = inputs["p1_b"].reshape(C, 1).astype(np.float32)
    # offset convs: reorder output channels to [y-taps, x-taps]
    ord1 = np.concatenate([np.arange(0, 2 * K1, 2), np.arange(1, 2 * K1, 2)])
    off0w = inputs["off0_w"][ord1]  # [50, 64, 5, 5]
    off0w = np.ascontiguousarray(off0w.transpose(1, 2, 3, 0).reshape(C, K1, 2 * K1))
    off0b = inputs["off0_b"][ord1].reshape(2 * K1, 1).astype(np.float32)
    ord2 = np.concatenate([np.arange(0, 2 * K2, 2), np.arange(1, 2 * K2, 2)])
    offsw = inputs["offs_w"][ord2]
    offsw = np.ascontiguousarray(offsw.transpose(1, 2, 3, 0).reshape(C, K2, 2 * K2))
    offsb = inputs["offs_b"][ord2].reshape(2 * K2, 1).astype(np.float32)
    dwk1 = np.broadcast_to(
        inputs["dw0_w"].reshape(C, K1).T[None, :, :], (128, K1, C)).astype(np.float32)
    dwk2 = np.broadcast_to(
        inputs["dws_w"].reshape(C, K2).T[None, :, :], (128, K2, C)).astype(np.float32)
    g1w = np.ascontiguousarray(inputs["g1_w"].T)
    g1b = inputs["g1_b"].reshape(C, 1).astype(np.float32)
    p2w = np.ascontiguousarray(inputs["p2_w"].T)
    p2b = inputs["p2_b"].reshape(C, 1).astype(np.float32)
    ident = np.eye(128, dtype=np.float32)

    def btiles(KK, dil, ksz, win_margin):
        ky = dil * (np.arange(KK) // ksz - (ksz // 2))
        kx = dil * (np.arange(KK) % ksz - (ksz // 2))
        p = np.arange(128)
        by = np.zeros((128, 4 * KK), np.float32)
        bx = np.zeros((128, 4 * KK), np.float32)
        for bb in range(4):
            rp, xh = bb // 2, bb % 2
            by[:, bb * KK:(bb + 1) * KK] = ky[None, :] + win_margin + rp
            bx[:, bb * KK:(bb + 1) * KK] = (kx[None, :] + p[:, None]
                                            + PADC + 128 * xh)
        return by, bx

    by1_, bx1_ = btiles(K1, 1, 5, WIN1)
    by2_, bx2_ = btiles(K2, 3, 7, WIN2)

    common = dict(cw3=cw3, p1w=p1w, p1b=p1b, off0w=off0w, off0b=off0b,
                  offsw=offsw, offsb=offsb, dwk1=dwk1, dwk2=dwk2,
                  g1w=g1w, g1b=g1b, p2w=p2w, p2b=p2b, identw=ident,
                  by1=by1_, bx1=bx1_, by2=by2_, bx2=bx2_)

    in_maps = []
    for core in range(N_CORES):
        b, half = core // 2, core % 2
        r0 = 128 * half
        xi = x[b]  # [C,H,W]
        x_full = np.zeros((C, H + 2, W + 2), np.float32)
        x_full[:, 1:-1, 1:-1] = xi
        # x_local rows: img rows r0-25 .. r0+152 (178 rows), cols pad 1
        x_local = np.zeros((C, NHROWS + 2, W + 2), np.float32)
        lo, hi = r0 - 25, r0 + 153
        vlo, vhi = max(lo, 0), min(hi, H)
        x_local[:, vlo - lo:vhi - lo, 1:-1] = xi[:, vlo:vhi, :]
        # hmask: group g covers local rows (2g-24, 2g-23)
        hmask = np.zeros((NHROWS // 2, C, 2 * W), np.float32)
        for g in range(NHROWS // 2):
            for rr in range(2):
                l = 2 * g + HL0 + rr
                if 0 <= r0 + l < H:
                    hmask[g, :, rr * W:(rr + 1) * W] = 1.0
        m = dict(common)
        m.update(x_full=x_full, x_local=x_local, hmask=hmask)
        in_maps.append(m)
    return in_maps


_CACHED = {}


LAST_EXEC_NS = None


def kernel(**inputs):
    global LAST_EXEC_NS
    if "nc" not in _CACHED:
        _CACHED["nc"] = build_program()
    nc = _CACHED["nc"]
    in_maps = prepare_inputs(inputs)
    trace = bool(int(os.environ.get("KERNEL_TRACE", "0")))
    res = run_bass_kernel_spmd(nc, in_maps, list(range(N_CORES)), trace=trace)
    if res.exec_time_ns is not None:
        LAST_EXEC_NS = res.exec_time_ns
    out = np.zeros((B, C, H, W), np.float32)
    for core in range(N_CORES):
        b, half = core // 2, core % 2
        out[b, :, 128 * half:128 * (half + 1), :] = res.results[core]["out"]
    return out


if __name__ == "__main__":
    import reference as R
    inp = {k: np.asarray(v) for k, v in R.setup_inputs().items()}
    o = kernel(**inp)
    ref = np.load("/root/problem/ref_out.npy")
    err = np.abs(o - ref).max() / (np.abs(ref).max() + 1e-9)
    print("rel err:", err)



# revision 9
# speedup vs baseline: 1.0051x; 1.0051x over previous
"""Trainium2 Bass kernel for nn_DLKAConvBlock (B=4, C=64, H=W=256) on 8 NeuronCores.

Sharding: data-parallel over (batch, H-half): core = 2*b + half, each core
computes output rows [r0, r0+128) of image b (r0 = 128*half), working in a
local row coordinate frame l (img row = r0 + l) so the SPMD program is
identical across cores; all per-core differences are carried by input data.

v2 pipeline per core (all on-device):
  phase1': conv3x3 over the non-local image rows (x_rest) - stats only
  phase2:  conv3x3 on local rows -> h_local, accumulating stats
  stats:   column-masked (gmask) reduce -> instance-norm scale/bias
  phase3:  t = mask*gelu(p1 @ norm(h)) -> t_nchw (f32) + t_int (bf16,
           row-interleaved NHWC: t_int[y][x][0:64]=t(y,x), [64:128]=t(y+1,x))
  deform stages: offset conv -> per-pixel-tap single-descriptor (512B)
           gathers from the interleaved table -> wide DVE combine
           (bigmult x quadrant weights, dwk mult) -> PE transpose-accumulate
           reduction over taps -> fold -> consumer.
  Gather indices are built in the SWDGE wrapped layout via PE selector
  matmuls (no DRAM bounce).
"""
import os
import sys
from contextlib import ExitStack

import numpy as np

for _p in ("/opt/trn_rl_repo", "/root/.axon_site/_ro/trn_rl_repo"):
    if os.path.isdir(_p) and _p not in sys.path:
        sys.path.insert(0, _p)

import concourse.bass as bass
import concourse.bacc as bacc
import concourse.mybir as mybir
from concourse import tile
from concourse.bass_utils import run_bass_kernel_spmd

F32 = mybir.dt.float32
BF16 = mybir.dt.bfloat16
I16 = mybir.dt.int16
ALU = mybir.AluOpType
ACTF = mybir.ActivationFunctionType
F32R = mybir.dt.float32r

B, C, H, W = 4, 64, 256, 256
EPS = 1e-5
N_CORES = 8

# local-frame regions
HL0, HL1 = -24, 152          # h_local / t rows
NHROWS = HL1 - HL0           # 176
A1L0, A1L1 = -16, 144        # a1 rows
NA1ROWS = A1L1 - A1L0        # 160
PADR = 32                    # interleaved-table row pad (array row = l + 32)
PADC = 16                    # interleaved-table col pad
NPR = 192                    # table rows: l in [-32, 160)
NPC = 288                    # table cols: x in [-16, 272)
K1, K2 = 25, 49
NREST = 52                   # phase1' groups (104 rows)
NSTAT = 144                  # stats columns (88 + 52, padded)

D1_CHUNKS = [(-16, 48), (48, 112), (112, 144)]
D2_CHUNKS = [(0, 64), (64, 128)]
WIN1 = 8    # deform1 window margin rows
WIN2 = 16   # deform2 window margin rows


def _ap_raw(t_handle, offset, pattern):
    """Build an AP with an explicit [step, count] pattern on a tensor handle."""
    return bass.AP(t_handle, offset, [list(p) for p in pattern])


def build_program():
    nc = bacc.Bacc("TRN2", target_bir_lowering=False, debug=False, enable_asserts=False)

    # ---------------- external inputs ----------------
    x_local = nc.declare_dram_parameter("x_local", [C, NHROWS + 2, W + 2], F32R, isOutput=False)
    x_rest = nc.declare_dram_parameter("x_rest", [C, 2 * NREST + 2, W + 2], F32R, isOutput=False)
    hmask2 = nc.declare_dram_parameter("hmask2", [NHROWS // 2, C, 2], F32, isOutput=False)
    gmaskw = nc.declare_dram_parameter("gmaskw", [C, NSTAT], F32, isOutput=False)
    cw3 = nc.declare_dram_parameter("cw3", [C, 9, C], F32R, isOutput=False)
    p1w = nc.declare_dram_parameter("p1w", [C, C], F32, isOutput=False)
    p1b = nc.declare_dram_parameter("p1b", [C, 1], F32, isOutput=False)
    off0w = nc.declare_dram_parameter("off0w", [C, K1, 2 * K1], F32R, isOutput=False)
    off0b = nc.declare_dram_parameter("off0b", [2 * K1, 1], F32, isOutput=False)
    offsw = nc.declare_dram_parameter("offsw", [C, K2, 2 * K2], F32R, isOutput=False)
    offsb = nc.declare_dram_parameter("offsb", [2 * K2, 1], F32, isOutput=False)
    dwk1 = nc.declare_dram_parameter("dwk1", [128, K1, C], BF16, isOutput=False)
    dwk2 = nc.declare_dram_parameter("dwk2", [128, K2, C], BF16, isOutput=False)
    g1w = nc.declare_dram_parameter("g1w", [C, C], F32, isOutput=False)
    g1b = nc.declare_dram_parameter("g1b", [C, 1], F32, isOutput=False)
    p2w = nc.declare_dram_parameter("p2w", [C, C], F32, isOutput=False)
    p2b = nc.declare_dram_parameter("p2b", [C, 1], F32, isOutput=False)
    identw = nc.declare_dram_parameter("identw", [128, 128], F32, isOutput=False)
    identb = nc.declare_dram_parameter("identb", [128, 128], BF16, isOutput=False)
    selw = nc.declare_dram_parameter("selw", [128, 8, 128], F32, isOutput=False)
    sel64 = nc.declare_dram_parameter("sel64", [128, C], F32, isOutput=False)
    by1 = nc.declare_dram_parameter("by1", [128, 4 * K1], F32, isOutput=False)
    bx1 = nc.declare_dram_parameter("bx1", [128, 4 * K1], F32, isOutput=False)
    by2 = nc.declare_dram_parameter("by2", [128, 4 * K2], F32, isOutput=False)
    bx2 = nc.declare_dram_parameter("bx2", [128, 4 * K2], F32, isOutput=False)

    out_t = nc.declare_dram_parameter("out", [C, 128, W], F32, isOutput=True)

    # ---------------- internal DRAM ----------------
    h_local = nc.dram_tensor("h_local", [C, NHROWS, W], F32)
    t_nchw = nc.dram_tensor("t_nchw", [C, NHROWS, W + 4], F32R)
    t_int = nc.dram_tensor("t_int", [NPR, NPC, 128], BF16)
    a1_nchw = nc.dram_tensor("a1_nchw", [C, NA1ROWS, W + 18], F32R)
    a1_int = nc.dram_tensor("a1_int", [NPR, NPC, 128], BF16)

    with tile.TileContext(nc) as tc, ExitStack() as ctx:
        PHASES = int(os.environ.get("KERNEL_PHASES", "5"))
        gather_regs = {n: nc.gpsimd.to_reg(n) for n in (K1 * 128, K2 * 128)}
        statics = ctx.enter_context(tc.tile_pool(name="statics", bufs=1))
        # resident static tiles
        s_cw3 = statics.tile([C, 9, C], F32R)
        s_p1w = statics.tile([C, C], F32)
        s_p1b = statics.tile([C, 1], F32)
        s_g1w = statics.tile([C, C], F32)
        s_g1b = statics.tile([C, 1], F32)
        s_p2w = statics.tile([C, C], F32)
        s_p2b = statics.tile([C, 1], F32)
        s_id = statics.tile([128, 128], F32)
        s_idb = statics.tile([128, 128], BF16)
        s_sel = statics.tile([128, 8, 128], F32)
        s_sel64 = statics.tile([128, C], F32)
        s_dw1 = statics.tile([128, K1, C], BF16)
        s_dw2 = statics.tile([128, K2, C], BF16)
        s_gmask = statics.tile([C, NSTAT], F32)
        s_hmask = statics.tile([C, NHROWS // 2, 2], F32)
        s_zero = statics.tile([128, 1024], F32)
        for dst, src in [(s_cw3, cw3), (s_p1w, p1w), (s_p1b, p1b),
                         (s_g1w, g1w), (s_g1b, g1b), (s_p2w, p2w), (s_p2b, p2b),
                         (s_id, identw), (s_idb, identb), (s_sel, selw),
                         (s_sel64, sel64), (s_dw1, dwk1), (s_dw2, dwk2),
                         (s_gmask, gmaskw)]:
            nc.sync.dma_start(dst[:], src[:])
        nc.sync.dma_start(s_hmask[:], hmask2[:].rearrange("g c r -> c g r"))
        nc.vector.memset(s_zero[:], 0.0)

        # stats accumulators
        s_sum = statics.tile([C, NSTAT], F32)
        s_sq = statics.tile([C, NSTAT], F32)
        nc.vector.memset(s_sum[:], 0.0)
        nc.vector.memset(s_sq[:], 0.0)
        s_rstd = statics.tile([C, 1], F32)
        s_nbias = statics.tile([C, 1], F32)   # -mean*rstd
        s_cb = statics.tile([C, 1], F32)      # p2b + nbias
        s_tmp1 = statics.tile([C, 1], F32)
        s_tmp2 = statics.tile([C, 1], F32)

        # ---------------- memset interleaved tables (pads) ----------------
        for dram in (t_int, a1_int):
            flat = dram[:].bitcast(F32).rearrange("a b c -> (a b c)")
            total = NPR * NPC * 64  # f32 elems
            CH = 128 * 1024
            pos = 0
            while pos < total:
                n = min(CH, total - pos)
                rows = n // 1024
                nc.scalar.dma_start(
                    flat[pos:pos + n].rearrange("(p f) -> p f", p=rows),
                    s_zero[:rows, :])
                pos += n
        # t_nchw/a1_nchw column pads (first+last cols) are covered by full
        # memset of those tensors (cheap relative to everything else).
        for dram, wpad in ((t_nchw, W + 4), (a1_nchw, W + 18)):
            flat = dram[:].bitcast(F32).rearrange("a b c -> (a b c)")
            total = int(np.prod(dram.shape))
            CH = 128 * 1024
            pos = 0
            while pos < total:
                n = min(CH, total - pos)
                rows = n // 1024
                if rows >= 1 and rows * 1024 == n:
                    nc.scalar.dma_start(
                        flat[pos:pos + n].rearrange("(p f) -> p f", p=rows),
                        s_zero[:rows, :])
                else:
                    nc.scalar.dma_start(flat[pos:pos + n], s_zero[0:1, :n])
                pos += n

        psum_conv = ctx.enter_context(
            tc.tile_pool(name="psum_conv", bufs=1, space="PSUM"))
        psum_tr = ctx.enter_context(
            tc.tile_pool(name="psum_tr", bufs=1, space="PSUM"))
        psum_wrp = ctx.enter_context(
            tc.tile_pool(name="psum_wrp", bufs=2, space="PSUM"))
        psum_acc = ctx.enter_context(
            tc.tile_pool(name="psum_acc", bufs=1, space="PSUM"))

        # ---------------- phase 1': conv3x3 stats on x_rest ----------------
        with tc.tile_pool(name="ph1", bufs=3) as ph1:
          if PHASES >= 1:
            for g in range(NREST):
                xt = ph1.tile([C, 4, W + 2], F32R, tag="xt")
                nc.sync.dma_start(xt[:], x_rest[:, 2 * g:2 * g + 4, :])
                ps = psum_conv.tile([C, 2 * W], F32, tag="conv")
                for k in range(9):
                    ky, kx = k // 3, k % 3
                    rhs = xt[:, ky:ky + 2, kx:kx + W]
                    nc.tensor.matmul(ps[:].rearrange("c (r w) -> c r w", r=2),
                                     s_cw3[:, k, :], rhs,
                                     start=(k == 0), stop=(k == 8))
                hd = ph1.tile([C, 2 * W], F32, tag="hd")
                nc.scalar.activation(hd[:], ps[:], ACTF.Copy,
                                     accum_out=s_sum[:, 88 + g:89 + g])
                sqd = ph1.tile([C, 2 * W], F32, tag="sqd")
                nc.scalar.activation(sqd[:], hd[:], ACTF.Square,
                                     accum_out=s_sq[:, 88 + g:89 + g])

        # ---------------- phase 2: h_local conv3x3 (+stats) ----------------
        with tc.tile_pool(name="ph2", bufs=3) as ph2:
          if PHASES >= 2:
            for g in range(NHROWS // 2):
                xt = ph2.tile([C, 4, W + 2], F32R, tag="xt")
                nc.sync.dma_start(xt[:], x_local[:, 2 * g:2 * g + 4, :])
                ps = psum_conv.tile([C, 2 * W], F32, tag="conv")
                for k in range(9):
                    ky, kx = k // 3, k % 3
                    nc.tensor.matmul(ps[:].rearrange("c (r w) -> c r w", r=2),
                                     s_cw3[:, k, :], xt[:, ky:ky + 2, kx:kx + W],
                                     start=(k == 0), stop=(k == 8))
                hsb = ph2.tile([C, 2 * W], F32, tag="hsb")
                nc.scalar.activation(hsb[:], ps[:], ACTF.Copy,
                                     accum_out=s_sum[:, g:g + 1])
                sqd = ph2.tile([C, 2 * W], F32, tag="sqd")
                nc.scalar.activation(sqd[:], hsb[:], ACTF.Square,
                                     accum_out=s_sq[:, g:g + 1])
                nc.scalar.dma_start(h_local[:, 2 * g:2 * g + 2, :],
                                    hsb[:].rearrange("c (r w) -> c r w", r=2))

        # finalize stats (column-masked)
        nc.vector.tensor_tensor(s_sum[:], s_sum[:], s_gmask[:], ALU.mult)
        nc.vector.tensor_tensor(s_sq[:], s_sq[:], s_gmask[:], ALU.mult)
        nc.vector.tensor_reduce(s_tmp1[:], s_sum[:], mybir.AxisListType.X, ALU.add)
        nc.vector.tensor_reduce(s_tmp2[:], s_sq[:], mybir.AxisListType.X, ALU.add)
        inv_n = 1.0 / (H * W)
        nc.vector.tensor_scalar(s_tmp1[:], s_tmp1[:], inv_n, None, ALU.mult)
        nc.vector.tensor_scalar(s_tmp2[:], s_tmp2[:], inv_n, None, ALU.mult)
        var = statics.tile([C, 1], F32)
        nc.vector.scalar_tensor_tensor(var[:], s_tmp1[:], s_tmp1[:], s_tmp2[:],
                                       ALU.mult, ALU.subtract)
        nc.vector.tensor_scalar(var[:], var[:], -1.0, EPS, ALU.mult, ALU.add)
        nc.scalar.sqrt(var[:], var[:])
        nc.vector.reciprocal(s_rstd[:], var[:])
        nc.vector.scalar_tensor_tensor(s_nbias[:], s_tmp1[:], -1.0, s_rstd[:],
                                       ALU.mult, ALU.mult)
        nc.vector.tensor_tensor(s_cb[:], s_p2b[:], s_nbias[:], ALU.add)

        # ---------------- phase 3: t = mask*gelu(p1 @ norm(h)) ----------------
        with tc.tile_pool(name="ph3", bufs=3) as ph3:
          if PHASES >= 3:
            for g in range(NHROWS // 2):
                hsb = ph3.tile([C, 2 * W], F32, tag="hld")
                nc.sync.dma_start(
                    hsb[:], h_local[:, 2 * g:2 * g + 2, :].rearrange("c r w -> c (r w)"))
                hn = ph3.tile([C, 2 * W], F32, tag="hn")
                nc.vector.tensor_scalar(hn[:], hsb[:], s_rstd[:], s_nbias[:],
                                        ALU.mult, ALU.add)
                ps = psum_conv.tile([C, 2 * W], F32, tag="conv")
                nc.tensor.matmul(ps[:], s_p1w[:], hn[:], start=True, stop=True)
                tt_ = ph3.tile([C, 2 * W], F32, tag="tt")
                nc.scalar.activation(tt_[:], ps[:], ACTF.Gelu, bias=s_p1b[:])
                tm = ph3.tile([C, 2 * W], F32, tag="tm")
                # mask: s_hmask[:, g, :] is [C, 2] (one value per row of the
                # pair); broadcast along W via a stride-0 inner dim.
                nc.vector.tensor_tensor(
                    tm[:].rearrange("c (r w) -> c r w", r=2),
                    tt_[:].rearrange("c (r w) -> c r w", r=2),
                    s_hmask[:, g, :].unsqueeze(2).broadcast_to((C, 2, W)),
                    ALU.mult)
                nc.scalar.dma_start(t_nchw[:, 2 * g:2 * g + 2, 2:2 + W].bitcast(F32),
                                    tm[:].rearrange("c (r w) -> c r w", r=2))
                # interleaved bf16 NHWC writes
                for bb in range(4):
                    pst_full = psum_tr.tile([128, 128], F32, tag="tr")
                    pst = pst_full[:, :C]
                    nc.tensor.matmul(pst[:], tm[:, 128 * bb:128 * (bb + 1)],
                                     s_id[:C, :C], start=True, stop=True,
                                     is_transpose=True)
                    tTb = ph3.tile([128, C], BF16, tag="tTb")
                    nc.vector.tensor_copy(tTb[:], pst[:])
                    l = 2 * g + HL0 + bb // 2
                    xh = bb % 2
                    r = l + PADR
                    c0 = PADC + 128 * xh
                    nc.scalar.dma_start(
                        t_int[r, c0:c0 + 128, 0:C], tTb[:])
                    nc.scalar.dma_start(
                        t_int[r - 1, c0:c0 + 128, C:128], tTb[:])

        # ---------------- deform stages ----------------
        def deform_stage(name, chunks, KK, d_by, d_bx, d_offw, d_offb, s_dwk,
                         src_nchw, src_int, win_margin, out_stage):
            """Emit one deformable depthwise conv stage (v2)."""
            Kg2 = 2 * KK
            NIDX = KK * 128
            with tc.tile_pool(name=name + "s", bufs=1) as st, \
                 tc.tile_pool(name=name, bufs=2) as dp, \
                 tc.tile_pool(name=name + "r", bufs=2) as rp_pool, \
                 tc.tile_pool(name=name + "g", bufs=2) as gp:
                s_offw = st.tile([C, KK, Kg2], F32R)
                s_offb = st.tile([Kg2, 1], F32)
                s_bw = st.tile([128, 4 * KK], F32)
                s_bxw = st.tile([128, 4 * KK], F32)
                for dst, src in [(s_offw, d_offw), (s_offb, d_offb),
                                 (s_bw, d_by), (s_bxw, d_bx)]:
                    nc.sync.dma_start(dst[:], src[:])
                for (c0, c1) in chunks:
                    win_l0 = c0 - win_margin
                    win_rows = (c1 - c0) + 2 * win_margin
                    n_units = win_rows * NPC
                    win_off = (win_l0 + PADR) * NPC * 128  # bf16 elems
                    for l0 in range(c0, c1, 2):
                        # ---- offset conv on rows (l0, l0+1) ----
                        if name == "d1":
                            rt = rp_pool.tile([C, 6, W + 4], F32R, tag="rt")
                            nc.sync.dma_start(
                                rt[:], src_nchw[:, (l0 - 2) - HL0:(l0 + 4) - HL0, :])
                            ps = psum_conv.tile([Kg2, 2 * W], F32, tag="conv")
                            for k in range(KK):
                                ky, kx = k // 5 - 2, k % 5 - 2
                                nc.tensor.matmul(
                                    ps[:].rearrange("c (r w) -> c r w", r=2),
                                    s_offw[:, k, :],
                                    rt[:, (ky + 2):(ky + 4), (kx + 2):(kx + 2) + W],
                                    start=(k == 0), stop=(k == KK - 1))
                        else:
                            rt = rp_pool.tile([C, 20, W + 18], F32R, tag="rt")
                            nc.sync.dma_start(
                                rt[:], src_nchw[:, (l0 - 9) - A1L0:(l0 + 11) - A1L0, :])
                            ps = psum_conv.tile([Kg2, 2 * W], F32, tag="conv")
                            for k in range(KK):
                                ky, kx = 3 * (k // 7 - 3), 3 * (k % 7 - 3)
                                nc.tensor.matmul(
                                    ps[:].rearrange("c (r w) -> c r w", r=2),
                                    s_offw[:, k, :],
                                    rt[:, (ky + 9):(ky + 11), (kx + 9):(kx + 9) + W],
                                    start=(k == 0), stop=(k == KK - 1))
                        osb = dp.tile([Kg2, 2 * W], F32, tag="osb")
                        nc.scalar.activation(osb[:], ps[:], ACTF.Identity, bias=s_offb[:])
                        # ---- transpose offsets to [px, ch] for 4 batches ----
                        offsT = dp.tile([128, 4 * Kg2], F32, tag="offsT")
                        for bb in range(4):
                            pst_full = psum_tr.tile([128, 128], F32, tag="tr")
                            pst = pst_full[:, :Kg2]
                            nc.tensor.matmul(pst[:], osb[:, 128 * bb:128 * (bb + 1)],
                                             s_id[:Kg2, :Kg2], start=True, stop=True,
                                             is_transpose=True)
                            nc.vector.tensor_copy(
                                offsT[:, Kg2 * bb:Kg2 * (bb + 1)], pst[:])
                        # ---- index & weight prep on [128, 4*KK] views ----
                        yv = offsT[:].rearrange("p (b c) -> p b c", b=4)[:, :, 0:KK]
                        xv = offsT[:].rearrange("p (b c) -> p b c", b=4)[:, :, KK:Kg2]
                        py = dp.tile([128, 4 * KK], F32, tag="py")
                        px = dp.tile([128, 4 * KK], F32, tag="px")
                        pyv = py[:].rearrange("p (b k) -> p b k", b=4)
                        pxv = px[:].rearrange("p (b k) -> p b k", b=4)
                        nc.vector.tensor_tensor(pyv, yv, s_bw[:].rearrange(
                            "p (b k) -> p b k", b=4), ALU.add)
                        nc.vector.tensor_scalar(py[:], py[:], float(l0 - win_l0),
                                                None, ALU.add)
                        nc.vector.tensor_tensor(pxv, xv, s_bxw[:].rearrange(
                            "p (b k) -> p b k", b=4), ALU.add)
                        MAGIC = 8388608.0
                        y0 = dp.tile([128, 4 * KK], F32, tag="y0")
                        x0 = dp.tile([128, 4 * KK], F32, tag="x0")
                        nc.vector.tensor_scalar(y0[:], py[:], MAGIC - 0.5,
                                                -MAGIC, ALU.add, ALU.add)
                        nc.vector.tensor_scalar(x0[:], px[:], MAGIC - 0.5,
                                                -MAGIC, ALU.add, ALU.add)
                        fy = dp.tile([128, 4 * KK], F32, tag="fy")
                        fx = dp.tile([128, 4 * KK], F32, tag="fx")
                        nc.vector.tensor_tensor(fy[:], py[:], y0[:], ALU.subtract)
                        nc.vector.tensor_tensor(fx[:], px[:], x0[:], ALU.subtract)
                        nc.vector.tensor_scalar(y0[:], y0[:], float(win_rows - 2),
                                                0.0, ALU.min, ALU.max)
                        nc.vector.tensor_scalar(x0[:], x0[:], float(NPC - 2),
                                                0.0, ALU.min, ALU.max)
                        idxf = dp.tile([128, 4 * KK], F32, tag="idxf")
                        nc.vector.scalar_tensor_tensor(idxf[:], y0[:], float(NPC),
                                                       x0[:], ALU.mult, ALU.add)
                        fyb = dp.tile([128, 4 * KK], F32, tag="fyb")
                        fxb = dp.tile([128, 4 * KK], F32, tag="fxb")
                        nc.vector.tensor_scalar(fyb[:], fy[:], -1.0, 1.0,
                                                ALU.mult, ALU.add)
                        nc.vector.tensor_scalar(fxb[:], fx[:], -1.0, 1.0,
                                                ALU.mult, ALU.add)
                        # quadrant weight tiles [128, KK, 4] per batch,
                        # col order matches elem: (V00, V10, V01, V11)
                        wt = dp.tile([128, 4, KK, 4], F32, tag="wt")
                        for bb in range(4):
                            sl = slice(bb * KK, (bb + 1) * KK)
                            nc.vector.tensor_tensor(
                                wt[:, bb, :, 0], fyb[:, sl], fxb[:, sl], ALU.mult)
                            nc.vector.tensor_tensor(
                                wt[:, bb, :, 1], fy[:, sl], fxb[:, sl], ALU.mult)
                            nc.vector.tensor_tensor(
                                wt[:, bb, :, 2], fyb[:, sl], fx[:, sl], ALU.mult)
                            nc.vector.tensor_tensor(
                                wt[:, bb, :, 3], fy[:, sl], fx[:, sl], ALU.mult)
                        # ---- wrapped idx via PE selector matmuls ----
                        wrapped = gp.tile([128, 4, KK, 8], I16, tag="wrp")
                        for q in range(8):
                            psw = psum_wrp.tile([128, 4 * KK], F32, tag="wrp")
                            nc.tensor.matmul(psw[:], s_sel[:, q, :], idxf[:],
                                             start=True, stop=True)
                            nc.vector.tensor_copy(
                                wrapped[:, :, :, q],
                                psw[:].rearrange("p (b k) -> p b k", b=4))
                        # ---- per batch: gather + combine ----
                        for bb in range(4):
                            rp, xh = bb // 2, bb % 2
                            gt = gp.tile([128, KK, 256], BF16, tag="gth")
                            inap = _ap_raw(
                                src_int, win_off, [[128, n_units], [1, 256]])
                            nc.gpsimd.dma_gather(
                                gt[:], inap,
                                wrapped[:, bb, :, :].rearrange("p k q -> p (k q)"),
                                NIDX, gather_regs[NIDX], 256, 128,
                                single_packet=False)
                            # bigmult (in place): gt *= wt (broadcast over ch)
                            gtv = gt[:].rearrange("p k (q c) -> p k q c", q=4)
                            nc.vector.tensor_tensor(
                                gtv, gtv,
                                wt[:, bb, :, :].unsqueeze(3).broadcast_to(
                                    (128, KK, 4, C)),
                                ALU.mult)
                            # dwk mult (in place, broadcast over quadrant)
                            nc.vector.tensor_tensor(
                                gtv, gtv,
                                s_dwk[:].unsqueeze(2).broadcast_to(
                                    (128, KK, 4, C)),
                                ALU.mult)
                            # PE reduce: accumulate quadrant-pairs over taps
                            accp = psum_tr.tile([128, 128], F32, tag="red")
                            for k in range(KK):
                                nc.tensor.matmul(
                                    accp[:],
                                    gt[:, k, 0:128],
                                    s_idb[:], start=(k == 0), stop=False)
                                nc.tensor.matmul(
                                    accp[:],
                                    gt[:, k, 128:256],
                                    s_idb[:], start=False, stop=(k == KK - 1))
                            # fold halves: acc64 = accp[0:64] + accp[64:128]
                            accs = dp.tile([128, 128], F32, tag="accs")
                            nc.vector.tensor_copy(accs[:], accp[:])
                            acc = psum_acc.tile([C, 128], F32, tag="acc")
                            nc.tensor.matmul(acc[:], s_sel64[:], accs[:],
                                             start=True, stop=True)
                            out_stage(l0, rp, xh, acc, dp)

        # ---------------- deform1 consumer: write a1 ----------------
        def a1_out(l0, rp, xh, acc, dpool):
            l = l0 + rp
            a1sb = dpool.tile([C, 128], F32, tag="a1sb")
            nc.scalar.activation(a1sb[:], acc[:], ACTF.Copy)
            nc.scalar.dma_start(
                a1_nchw[:, l - A1L0,
                        9 + 128 * xh: 9 + 128 * (xh + 1)].bitcast(F32), a1sb[:])
            pst_full = psum_tr.tile([128, 128], F32, tag="tr")
            pst = pst_full[:, :C]
            nc.tensor.matmul(pst[:], a1sb[:], s_id[:C, :C], start=True, stop=True,
                             is_transpose=True)
            a1Tb = dpool.tile([128, C], BF16, tag="a1Tb")
            nc.vector.tensor_copy(a1Tb[:], pst[:])
            r = l + PADR
            c0 = PADC + 128 * xh
            nc.scalar.dma_start(a1_int[r, c0:c0 + 128, 0:C], a1Tb[:])
            nc.scalar.dma_start(a1_int[r - 1, c0:c0 + 128, C:128], a1Tb[:])

        def dump_to_out(src_dram, row_off, col_off, row_len):
            with tc.tile_pool(name="dump", bufs=2) as dmp:
                for g in range(64):
                    tl = dmp.tile([C, 2, W], F32, tag="dt")
                    nc.sync.dma_start(
                        tl[:], src_dram[:, row_off + 2 * g:row_off + 2 * g + 2,
                                        col_off:col_off + W].bitcast(F32))
                    nc.scalar.dma_start(out_t[:, 2 * g:2 * g + 2, :], tl[:])

        if PHASES >= 4:
            deform_stage("d1", D1_CHUNKS, K1, by1, bx1, off0w, off0b,
                         s_dw1, t_nchw, t_int, WIN1, a1_out)

        # ---------------- deform2 consumer: tail fusion ----------------
        def tail_out(l0, rp, xh, acc, dpool):
            l = l0 + rp
            a2sb = dpool.tile([C, 128], F32, tag="a2sb")
            nc.scalar.activation(a2sb[:], acc[:], ACTF.Copy)
            psg = psum_tr.tile([C, 128], F32, tag="tail")
            nc.tensor.matmul(psg[:], s_g1w[:], a2sb[:], start=True, stop=True)
            ut = dpool.tile([C, 128], F32, tag="ut")
            nc.sync.dma_start(
                ut[:], t_nchw[:, l - HL0,
                              2 + 128 * xh: 2 + 128 * (xh + 1)].bitcast(F32))
            t2 = dpool.tile([C, 128], F32, tag="t2")
            nc.vector.scalar_tensor_tensor(t2[:], psg[:], s_g1b[:], ut[:],
                                           ALU.add, ALU.mult)
            psp = psum_tr.tile([C, 128], F32, tag="tail")
            nc.tensor.matmul(psp[:], s_p2w[:], t2[:], start=True, stop=True)
            ht = dpool.tile([C, 128], F32, tag="ht")
            nc.sync.dma_start(
                ht[:], h_local[:, l - HL0, 128 * xh: 128 * (xh + 1)])
            v1 = dpool.tile([C, 128], F32, tag="v1")
            nc.scalar.activation(v1[:], psp[:], ACTF.Identity, bias=s_cb[:])
            v2 = dpool.tile([C, 128], F32, tag="v2")
            nc.vector.scalar_tensor_tensor(v2[:], ht[:], s_rstd[:], v1[:],
                                           ALU.mult, ALU.add)
            v3 = dpool.tile([C, 128], F32, tag="v3")
            nc.vector.scalar_tensor_tensor(v3[:], v2[:], 0.2, v2[:],
                                           ALU.mult, ALU.max)
            nc.scalar.dma_start(out_t[:, l, 128 * xh: 128 * (xh + 1)], v3[:])

        if PHASES >= 5:
            deform_stage("d2", D2_CHUNKS, K2, by2, bx2, offsw, offsb,
                         s_dw2, a1_nchw, a1_int, WIN2, tail_out)
        elif PHASES == 2:
            dump_to_out(h_local, -HL0, 0, 128)
        elif PHASES == 3:
            dump_to_out(t_nchw, -HL0, 2, 128)
        elif PHASES == 4:
            dump_to_out(a1_nchw, -A1L0, 9, 128)
        elif PHASES <= 1:
            dump_to_out(h_local, -HL0, 0, 128)

    nc.compile()
    return nc


def prepare_inputs(inputs):
    """Host-side marshaling: returns in_maps (list of 8 dicts)."""
    import ml_dtypes
    bf16 = ml_dtypes.bfloat16
    x = inputs["x"].astype(np.float32)
    conv_w = inputs["conv_w"].astype(np.float32)

    cw3 = np.ascontiguousarray(conv_w.transpose(1, 2, 3, 0).reshape(C, 9, C))
    p1w = np.ascontiguousarray(inputs["p1_w"].T)
    p1b = inputs["p1_b"].reshape(C, 1).astype(np.float32)
    ord1 = np.concatenate([np.arange(0, 2 * K1, 2), np.arange(1, 2 * K1, 2)])
    off0w = inputs["off0_w"][ord1]
    off0w = np.ascontiguousarray(off0w.transpose(1, 2, 3, 0).reshape(C, K1, 2 * K1))
    off0b = inputs["off0_b"][ord1].reshape(2 * K1, 1).astype(np.float32)
    ord2 = np.concatenate([np.arange(0, 2 * K2, 2), np.arange(1, 2 * K2, 2)])
    offsw = inputs["offs_w"][ord2]
    offsw = np.ascontiguousarray(offsw.transpose(1, 2, 3, 0).reshape(C, K2, 2 * K2))
    offsb = inputs["offs_b"][ord2].reshape(2 * K2, 1).astype(np.float32)
    dwk1 = np.broadcast_to(
        inputs["dw0_w"].reshape(C, K1).T[None, :, :], (128, K1, C)).astype(bf16)
    dwk2 = np.broadcast_to(
        inputs["dws_w"].reshape(C, K2).T[None, :, :], (128, K2, C)).astype(bf16)
    g1w = np.ascontiguousarray(inputs["g1_w"].T)
    g1b = inputs["g1_b"].reshape(C, 1).astype(np.float32)
    p2w = np.ascontiguousarray(inputs["p2_w"].T)
    p2b = inputs["p2_b"].reshape(C, 1).astype(np.float32)
    ident = np.eye(128, dtype=np.float32)
    identb = np.eye(128, dtype=np.float32).astype(bf16)
    # selector matrices: sel_q[p, i] = 1 if p == q*16 + (i % 16)
    selw = np.zeros((128, 8, 128), np.float32)
    for q in range(8):
        for i in range(128):
            selw[q * 16 + (i % 16), q, i] = 1.0
    # fold selector: sel64[p, c] = 1 if p % 64 == c
    sel64 = np.zeros((128, C), np.float32)
    for p in range(128):
        sel64[p, p % C] = 1.0

    def btiles(KK, dil, ksz, win_margin):
        ky = dil * (np.arange(KK) // ksz - (ksz // 2))
        kx = dil * (np.arange(KK) % ksz - (ksz // 2))
        p = np.arange(128)
        by = np.zeros((128, 4 * KK), np.float32)
        bx = np.zeros((128, 4 * KK), np.float32)
        for bb in range(4):
            rp, xh = bb // 2, bb % 2
            by[:, bb * KK:(bb + 1) * KK] = ky[None, :] + win_margin + rp
            bx[:, bb * KK:(bb + 1) * KK] = (kx[None, :] + p[:, None]
                                            + PADC + 128 * xh)
        return by, bx

    by1_, bx1_ = btiles(K1, 1, 5, WIN1)
    by2_, bx2_ = btiles(K2, 3, 7, WIN2)

    common = dict(cw3=cw3, p1w=p1w, p1b=p1b, off0w=off0w, off0b=off0b,
                  offsw=offsw, offsb=offsb, dwk1=dwk1, dwk2=dwk2,
                  g1w=g1w, g1b=g1b, p2w=p2w, p2b=p2b, identw=ident,
                  identb=identb, selw=selw, sel64=sel64,
                  by1=by1_, bx1=bx1_, by2=by2_, bx2=bx2_)

    in_maps = []
    for core in range(N_CORES):
        b, half = core // 2, core % 2
        r0 = 128 * half
        xi = x[b]  # [C,H,W]
        # x_local rows: img rows r0-25 .. r0+152 (178 rows), cols pad 1
        x_local = np.zeros((C, NHROWS + 2, W + 2), np.float32)
        lo, hi = r0 - 25, r0 + 153
        vlo, vhi = max(lo, 0), min(hi, H)
        x_local[:, vlo - lo:vhi - lo, 1:-1] = xi[:, vlo:vhi, :]
        # x_rest: the complementary 104 rows + halo
        x_rest = np.zeros((C, 2 * NREST + 2, W + 2), np.float32)
        if half == 0:
            # rest rows [152, 256); halo row 151; x_rest row j = img 151+j
            x_rest[:, 0:105, 1:-1] = xi[:, 151:256, :]
        else:
            # rest rows [0, 104); x_rest row 0 = img -1 (zero), j = img j-1
            x_rest[:, 1:105, 1:-1] = xi[:, 0:104, :]
        # hmask2: [88, C, 2] row-validity of local rows
        hmask2 = np.zeros((NHROWS // 2, C, 2), np.float32)
        for g in range(NHROWS // 2):
            for rr in range(2):
                l = 2 * g + HL0 + rr
                if 0 <= r0 + l < H:
                    hmask2[g, :, rr] = 1.0
        # gmask: stats column validity [C, NSTAT]
        gmask = np.zeros((C, NSTAT), np.float32)
        for g in range(NHROWS // 2):
            ok = True
            for rr in range(2):
                l = 2 * g + HL0 + rr
                ir = r0 + l
                if not (0 <= ir < H) and ir in (-1, H):
                    ok = False
            gmask[:, g] = 1.0 if ok else 0.0
        gmask[:, 88:88 + NREST] = 1.0
        m = dict(common)
        m.update(x_local=x_local, x_rest=x_rest, hmask2=hmask2, gmaskw=gmask)
        in_maps.append(m)
    return in_maps


_CACHED = {}


LAST_EXEC_NS = None


def kernel(**inputs):
    global LAST_EXEC_NS
    if "nc" not in _CACHED:
        _CACHED["nc"] = build_program()
    nc = _CACHED["nc"]
    in_maps = prepare_inputs(inputs)
    trace = bool(int(os.environ.get("KERNEL_TRACE", "0")))
    res = run_bass_kernel_spmd(nc, in_maps, list(range(N_CORES)), trace=trace)
    if res.exec_time_ns is not None:
        LAST_EXEC_NS = res.exec_time_ns
    out = np.zeros((B, C, H, W), np.float32)
    for core in range(N_CORES):
        b, half = core // 2, core % 2
        out[b, :, 128 * half:128 * (half + 1), :] = res.results[core]["out"]
    return out


if __name__ == "__main__":
    import reference as R
    inp = {k: np.asarray(v) for k, v in R.setup_inputs().items()}
    o = kernel(**inp)
    ref = np.load("/root/problem/ref_out.npy")
    err = np.abs(o - ref).max() / (np.abs(ref).max() + 1e-9)
    print("rel err:", err)
